# revision 43
# baseline (speedup 1.0000x reference)
"""Trainium2 Bass kernel for per-skill actor-critic MoE routing.

Expert-parallel: tokens are sorted by skill on the host, each skill's
group padded to capacity C, two skills per NeuronCore. Each core runs
its tokens through the per-skill actor/critic MLPs in a feature-major
layout (features on SBUF partitions, tokens on the free dim):

  L1: psum = [W1; b1].T/a @ [x.T; 1]      (K=65, fp32r, bias+1/a folded)
  h1 = tanh(a * psum)                     (ScalarE tanh, or a 2-pass
       custom-DVE polynomial composite with max err ~2e-3, to split the
       activation load across both engines)
  L2: psum = W2.T @ h1                    (K=128, fp32r)
  h2 = tanh(psum + b2)                    (ScalarE, bias via ACT bias AP)
  L3: per 512-token tile j, two accumulating matmuls with zero-padded
      [H, 36] weight blocks stack logits+value of up to 2 tiles into one
      [36, 512] psum block -> one narrow copy + DMA per span.

Host adds the L3 output bias and scatters back to original token order.
"""

import numpy as np

B, D, S, H, A = 32768, 64, 16, 128, 17
NCORES = 8
TILE = 512
SPAN = 1024
G = A + 1
KT = SPAN // TILE
M3 = KT * G
M6 = 2 * M3

# composite-tanh params (max abs err 2.0e-3 vs tanh, inputs prescaled 1/a)
A_SCALE = 3.7599417433556486
B0 = 2.8540707547137547
B1 = -3.003233876981482
B2 = 1.438174591155331
E0 = 1.304379694345473
E1 = -0.5021730968337124
E2 = 0.11673136204346712

_compiled = {}
_tanh_ops = None
VARIANT = "C"
DVE_H1_OVERRIDE = None
NWARM = 6
SPLIT_H1 = False
SPLIT_H1_UNITS = ()
UNIT_ORDER = "rag_both_last"


def _register_tanh_ops():
    """Register the 2-pass composite-tanh custom DVE ops (idempotent).

    pass1: w = uc*(C3 + y*(C2 + y*C1)), uc = clamp(Src0, C0, 1), y = uc^2
    pass2: v = clamp(Src0*(C0 + z*(C2 + z*C1)), C3, 1), z = Src0^2
    (C3 rides in1 as a [P,1] broadcast; exactly 8 ALU slices each.)
    """
    global _tanh_ops
    if _tanh_ops is not None:
        return _tanh_ops

    from concourse import dve_ops
    from concourse.dve_spec import (
        Spec, Src0, C0, C1, C2, C3, One, minn, maxx, lower,
    )
    from concourse.dve_ops import DveOp, _spill_c3_to_src1
    from concourse.dve_uop import DveOpSpec

    def _p1_ref(in0, in1, s0, s1, imm2):
        u = np.minimum(in0.astype(np.float32), 1.0)
        u = np.maximum(u, s0)
        y = u * u
        return u * (in1 + y * (imm2 + y * s1))

    def _p2_ref(in0, in1, s0, s1, imm2):
        w = in0.astype(np.float32)
        z = w * w
        v = w * (s0 + z * (imm2 + z * s1))
        return np.maximum(np.minimum(v, 1.0), in1)

    _uc = maxx(minn(Src0, One), C0)
    _y = _uc * _uc
    p1_body = _uc * (C3 + _y * (C2 + _y * C1))
    _z = Src0 * Src0
    p2_body = maxx(minn(Src0 * (C0 + _z * (C2 + _z * C1)), One), C3)

    ops = []
    for name, body, ref in (
        ("TANH_P1_ANT", p1_body, _p1_ref),
        ("TANH_P2_ANT", p2_body, _p2_ref),
    ):
        if name in dve_ops.CUSTOM_DVE_SPECS:
            ops.append(next(o for o in dve_ops.OPS if o.name == name))
            continue
        spec = Spec(body=_spill_c3_to_src1(body), reference=ref)
        opcode = dve_ops._CUSTOM_DVE_ROW_BASE + len(dve_ops.OPS)
        shas = {}
        for ver in ("v3", "v4"):
            try:
                sp = DveOpSpec(name=name, opcode=opcode,
                               uops=lower(spec, ver=ver), rd1_en=True)
                shas[ver] = sp.sha(ver)
            except Exception:
                pass
        op = DveOp(name, spec, subdim=False, uops_sha=shas)
        dve_ops.OPS.append(op)
        dve_ops.CUSTOM_DVE_SPECS[name] = spec
        dve_ops._SUB_OPCODE_FOR_NAME[name] = opcode
        ops.append(op)
    _tanh_ops = tuple(ops)
    return _tanh_ops


def _spans_for(C):
    spans = []
    off = 0
    while off < C:
        w = min(SPAN, C - off)
        spans.append((off, w))
        off += w
    return spans


def _tiles_for(W):
    tiles = []
    o = 0
    while o < W:
        w = min(TILE, W - o)
        tiles.append((o, w))
        o += w
    return tiles


def _build(C):
    """Build + compile the SPMD Tile kernel for per-skill capacity C."""
    import concourse.mybir as mybir
    import concourse.tile as tile
    from concourse import bacc

    P1, P2 = _register_tanh_ops()

    f32 = mybir.dt.float32
    f32r = mybir.dt.float32r
    Tanh = mybir.ActivationFunctionType.Tanh

    nc = bacc.Bacc("TRN2", target_bir_lowering=False, debug=False,
                   num_devices=NCORES)

    xin = nc.dram_tensor("xin", [D + 1, 2 * C], f32r, kind="ExternalInput")
    l1w = nc.dram_tensor("l1w", [D + 1, 4 * H], f32r, kind="ExternalInput")
    l2w = nc.dram_tensor("l2w", [H, 4 * H], f32r, kind="ExternalInput")
    l2b = nc.dram_tensor("l2b", [H, 4], f32, kind="ExternalInput")
    l3w = nc.dram_tensor("l3w", [H, 2 * 2 * KT * M6], f32r,
                         kind="ExternalInput")

    spans = _spans_for(C)
    out = nc.dram_tensor("out", [M6, len(spans), TILE], f32,
                         kind="ExternalOutput")

    # Unit order: small ragged spans first (fast pipeline fill) and last
    # (fast drain); full spans of the two skills interleaved in between.
    full = [(s, sp) for sp, (o, w) in enumerate(spans) if w == SPAN
            for s in (0, 1)]
    full.sort(key=lambda t: (t[1], t[0]))
    ragged = [(s, sp) for sp, (o, w) in enumerate(spans) if w < SPAN
              for s in (0, 1)]
    if UNIT_ORDER == "rag_first_last":
        units = ragged[:1] + full + ragged[1:]
    elif UNIT_ORDER == "rag_both_last":
        units = full + ragged
    else:
        units = ragged + full

    # h1 of these (skill, span, net) triples runs on the DVE composite;
    # the rest (and all h2) use exact ScalarE tanh.  Chosen to balance
    # ScalarE vs Vector engine busy time (~40% of h1 on DVE).
    if DVE_H1_OVERRIDE is not None:
        dve_h1 = set(DVE_H1_OVERRIDE(full, ragged))
    else:
        dve_h1 = set()
        for i, (s, sp) in enumerate(full):
            dve_h1.add((s, sp, i % 2))
        if ragged:
            s, sp = ragged[0]
            dve_h1 |= {(s, sp, 0), (s, sp, 1)}
        if len(ragged) > 1:
            dve_h1.add((ragged[1][0], ragged[1][1], 0))
    split_h1 = set()
    if SPLIT_H1:
        for i, (s_, sp_) in enumerate(full):
            if i in SPLIT_H1_UNITS:
                split_h1.add((s_, sp_, 1 - i % 2))

    with tile.TileContext(nc) as tc:
        with (
            tc.tile_pool(name="w", bufs=1) as wpool,
            tc.tile_pool(name="x", bufs=6) as xpool,
            tc.tile_pool(name="h", bufs=4) as hpool,
            tc.tile_pool(name="o", bufs=6) as opool,
            tc.tile_pool(name="ps", bufs=1, space="PSUM") as pspool,
        ):
            # L1 weights first (Pool queue, issues immediately); the first
            # x span rides the sync queue in parallel.
            l1w_sb = wpool.tile([D + 1, 4 * H], f32r)
            nc.gpsimd.dma_start(l1w_sb[:], l1w[:])
            l2w_sb = wpool.tile([H, 4 * H], f32r)
            nc.gpsimd.dma_start(l2w_sb[:], l2w[:])
            l2b_sb = wpool.tile([H, 4], f32)
            nc.gpsimd.dma_start(l2b_sb[:], l2b[:])
            l3w_sb = wpool.tile([H, 2 * 2 * KT * M6], f32r)
            nc.gpsimd.dma_start(l3w_sb[:], l3w[:])
            # [P,1] constants for the custom ops' spilled C3 operand
            cconst = wpool.tile([H, 2], f32)
            nc.gpsimd.memset(cconst[:, 0:1], B0)
            nc.gpsimd.memset(cconst[:, 1:2], -1.0)

            # PE warmup during the DMA-bound head: zero matmuls ramp the
            # HAM clock so real matmuls start at full speed.
            zt = wpool.tile([H, TILE], mybir.dt.bfloat16)
            nc.vector.memset(zt[:], 0.0)
            # dummy tanh: forces the ACT table load during the DMA head
            ztanh = wpool.tile([H, 1], f32)
            nc.scalar.activation(ztanh[:], cconst[:, 1:2], Tanh)
            for wi in range(NWARM):
                if VARIANT == "F":
                    wps = pspool.tile([H, TILE], f32, tag="l1", bufs=3)
                elif VARIANT in ("A", "C"):
                    wps = pspool.tile([H, SPAN], f32, tag="l1", bufs=2)
                elif VARIANT == "E":
                    wps = pspool.tile([H, SPAN], f32, tag="l2", bufs=3)
                else:
                    wps = pspool.tile([M3, TILE], f32, tag="l3", bufs=2)
                nc.tensor.matmul(wps[0:16, 0:TILE], zt[:, 0:16], zt[:])

            pair_ps = None
            for s, sp in units:
                off, W = spans[sp]
                tiles = _tiles_for(W)
                col0 = s * C + off
                slot = s * len(spans) + sp

                xt = xpool.tile([D + 1, SPAN], f32r, tag="x")
                nc.sync.dma_start(xt[:, :W], xin[:, col0:col0 + W])

                def do_l1(net):
                    if VARIANT == "F":
                        l1ps = []
                        lw = l1w_sb[:, (2 * s + net) * H:
                                    (2 * s + net + 1) * H]
                        for (to, tw) in tiles:
                            ps = pspool.tile([H, TILE], f32, tag="l1",
                                             bufs=3, name=f"l1ps{net}_{to}")
                            nc.tensor.matmul(ps[:, :tw], lw,
                                             xt[:, to:to + tw])
                            l1ps.append(ps)
                        return l1ps
                    l1ps = pspool.tile(
                        [H, SPAN], f32, tag="l1",
                        bufs=1 if VARIANT in ("B", "E") else 2,
                        name=f"l1ps{net}")
                    lw = l1w_sb[:, (2 * s + net) * H:(2 * s + net + 1) * H]
                    for (to, tw) in tiles:
                        nc.tensor.matmul(l1ps[:, to:to + tw], lw,
                                         xt[:, to:to + tw])
                    return l1ps

                def do_h1(net, l1ps):
                    h1 = hpool.tile([H, SPAN], f32r, tag="h1", name=f"h1_{net}")
                    mode = ("DVE" if (s, sp, net) in dve_h1 else
                            "SPLIT" if (s, sp, net) in split_h1 else "ACT")
                    if VARIANT == "F":
                        for ps, (to, tw) in zip(l1ps, tiles):
                            if mode == "DVE":
                                wt = hpool.tile([H, TILE], f32, tag="wt")
                                nc.vector._custom_dve(
                                    P1, out=wt[:, :tw], in0=ps[:, :tw],
                                    in1=cconst[:, 0:1], s0=-1.0, s1=B2,
                                    imm2=B1)
                                nc.vector._custom_dve(
                                    P2, out=h1[:, to:to + tw],
                                    in0=wt[:, :tw],
                                    in1=cconst[:, 1:2], s0=E0, s1=E2,
                                    imm2=E1)
                            else:
                                nc.scalar.activation(
                                    h1[:, to:to + tw], ps[:, :tw],
                                    Tanh, scale=float(A_SCALE))
                        return h1
                    if mode == "SPLIT" and len(tiles) > 1:
                        dv = tiles[0][1]
                        wt = hpool.tile([H, TILE], f32, tag="wt")
                        nc.vector._custom_dve(
                            P1, out=wt[:, :dv], in0=l1ps[:, :dv],
                            in1=cconst[:, 0:1], s0=-1.0, s1=B2, imm2=B1)
                        nc.vector._custom_dve(
                            P2, out=h1[:, :dv], in0=wt[:, :dv],
                            in1=cconst[:, 1:2], s0=E0, s1=E2, imm2=E1)
                        nc.scalar.activation(h1[:, dv:W], l1ps[:, dv:W],
                                             Tanh, scale=float(A_SCALE))
                    elif mode == "DVE":
                        wt = hpool.tile([H, SPAN], f32, tag="wtf")
                        nc.vector._custom_dve(
                            P1, out=wt[:, :W], in0=l1ps[:, :W],
                            in1=cconst[:, 0:1], s0=-1.0, s1=B2, imm2=B1)
                        nc.vector._custom_dve(
                            P2, out=h1[:, :W], in0=wt[:, :W],
                            in1=cconst[:, 1:2], s0=E0, s1=E2, imm2=E1)
                    else:
                        nc.scalar.activation(h1[:, :W], l1ps[:, :W],
                                             Tanh, scale=float(A_SCALE))
                    return h1

                def do_l2(net, h1):
                    if VARIANT == "F":
                        l2ps = pspool.tile([H, SPAN], f32, tag="l2",
                                           bufs=2, name=f"l2ps{net}")
                    elif VARIANT == "E":
                        l2ps = pspool.tile([H, SPAN], f32, tag="l2",
                                           bufs=3, name=f"l2ps{net}")
                    elif VARIANT == "A":
                        l2ps = pspool.tile([H, SPAN], f32, tag="l2",
                                           bufs=2, name=f"l2ps{net}")
                    elif VARIANT == "C":
                        l2ps = []
                        lw = l2w_sb[:, (2 * s + net) * H:
                                    (2 * s + net + 1) * H]
                        for (to, tw) in tiles:
                            ps = pspool.tile([H, TILE], f32, tag="l2",
                                             bufs=3, name=f"l2ps{net}_{to}")
                            nc.tensor.matmul(ps[:, :tw], lw,
                                             h1[:, to:to + tw])
                            l2ps.append(ps)
                        return l2ps
                    else:
                        l2ps = pspool.tile([H, SPAN], f32,
                                           tag="l2a" if net == 0 else "l2c",
                                           name=f"l2ps{net}")
                    lw = l2w_sb[:, (2 * s + net) * H:(2 * s + net + 1) * H]
                    for (to, tw) in tiles:
                        nc.tensor.matmul(l2ps[:, to:to + tw], lw,
                                         h1[:, to:to + tw])
                    return l2ps

                def do_h2(net, l2ps):
                    h2 = hpool.tile([H, SPAN], f32r, tag="h2", name=f"h2_{net}")
                    if VARIANT == "F":
                        nc.scalar.activation(
                            h2[:, :W], l2ps[:, :W], Tanh,
                            bias=l2b_sb[:, 2 * s + net:2 * s + net + 1])
                    elif VARIANT == "C":
                        for ps, (to, tw) in zip(l2ps, tiles):
                            nc.scalar.activation(
                                h2[:, to:to + tw], ps[:, :tw], Tanh,
                                bias=l2b_sb[:, 2 * s + net:2 * s + net + 1])
                    else:
                        nc.scalar.activation(
                            h2[:, :W], l2ps[:, :W], Tanh,
                            bias=l2b_sb[:, 2 * s + net:2 * s + net + 1])
                    return h2

                h2s = []
                if VARIANT in ("C", "F"):
                    psa = do_l1(0)
                    psc = do_l1(1)
                    h1a = do_h1(0, psa)
                    h1c = do_h1(1, psc)
                    l2a = do_l2(0, h1a)
                    l2c = do_l2(1, h1c)
                    h2s = [do_h2(0, l2a), do_h2(1, l2c)]
                    if pair_ps is None:
                        pair_ps = pspool.tile([M6, TILE], f32, tag="l3",
                                              bufs=1)
                        pair_first = True
                    else:
                        pair_first = False
                    l3tgt = pair_ps[:]
                elif VARIANT == "E":
                    psa = do_l1(0)
                    psc = do_l1(1)
                    h1a = do_h1(0, psa)
                    h1c = do_h1(1, psc)
                    l2a = do_l2(0, h1a)
                    l2c = do_l2(1, h1c)
                    h2s = [do_h2(0, l2a), do_h2(1, l2c)]
                    l3tgt = l2a[0:M3, 0:TILE]
                else:
                    l2s = []
                    for net in range(2):
                        ps = do_l1(net)
                        h1 = do_h1(net, ps)
                        l2ps = do_l2(net, h1)
                        h2s.append(do_h2(net, l2ps))
                        l2s.append(l2ps)
                    l3full = pspool.tile([M3, TILE], f32, tag="l3", bufs=2)
                    l3tgt = l3full[:]
                n_mm = 2 * len(tiles)
                mi = 0
                for net in range(2):
                    for j, (to, tw) in enumerate(tiles):
                        blk = ((s * 2 + net) * KT + j) * M6
                        nc.tensor.matmul(
                            l3tgt[:, 0:tw],
                            l3w_sb[:, blk:blk + M6],
                            h2s[net][:, to:to + tw],
                            start=(pair_first and mi == 0),
                            stop=(not pair_first and mi == n_mm - 1),
                            skip_group_check=True,
                        )
                        mi += 1
                if not pair_first:
                    osb = opool.tile([M6, TILE], f32, tag="o")
                    tw0 = tiles[0][1]
                    nc.vector.tensor_copy(osb[:, :tw0], l3tgt[:, 0:tw0])
                    nc.sync.dma_start(out[:, sp, :tw0], osb[:, :tw0])
                    pair_ps = None

    nc.compile()
    return nc


def _get_kernel(C):
    if C not in _compiled:
        _compiled[C] = _build(C)
    return _compiled[C]


def kernel(obs, skill_ids, Wa1, ba1, Wa2, ba2, Wa3, ba3,
           Wc1, bc1, Wc2, bc2, Wc3, bc3):
    from concourse.bass_utils import run_bass_kernel_spmd

    obs = np.asarray(obs, dtype=np.float32)
    sids = np.asarray(skill_ids).astype(np.int64)
    Wa1, ba1, Wa2, ba2, Wa3, ba3 = [np.asarray(a, np.float32)
                                    for a in (Wa1, ba1, Wa2, ba2, Wa3, ba3)]
    Wc1, bc1, Wc2, bc2, Wc3, bc3 = [np.asarray(a, np.float32)
                                    for a in (Wc1, bc1, Wc2, bc2, Wc3, bc3)]

    counts = np.bincount(sids, minlength=S)
    order = np.argsort(sids, kind="stable")
    starts = np.zeros(S + 1, np.int64)
    starts[1:] = np.cumsum(counts)

    C = max(2176, int(-(-counts.max() // 128) * 128))
    nc = _get_kernel(C)
    spans = _spans_for(C)

    inv_a = np.float32(1.0 / A_SCALE)
    obsT = np.ascontiguousarray(obs.T)  # [D, B]

    in_maps = []
    for c in range(NCORES):
        xin = np.zeros((D + 1, 2 * C), np.float32)
        l1w = np.zeros((D + 1, 4 * H), np.float32)
        l2w = np.zeros((H, 4 * H), np.float32)
        l2b = np.zeros((H, 4), np.float32)
        l3w = np.zeros((H, 2 * 2 * KT * M6), np.float32)
        for sloc in range(2):
            skill = 2 * c + sloc
            cnt = int(counts[skill])
            toks = order[starts[skill]:starts[skill] + cnt]
            xin[:D, sloc * C:sloc * C + cnt] = obsT[:, toks]
            xin[D, sloc * C:sloc * C + cnt] = 1.0
            for net, (W1, b1, W2, b2) in enumerate(
                ((Wa1, ba1, Wa2, ba2), (Wc1, bc1, Wc2, bc2))
            ):
                blk = slice((2 * sloc + net) * H, (2 * sloc + net + 1) * H)
                l1w[:D, blk] = W1[skill] * inv_a
                l1w[D, blk] = b1[skill] * inv_a
                l2w[:, blk] = W2[skill]
                l2b[:, 2 * sloc + net] = b2[skill]
            for j in range(KT):
                ro = (sloc * KT + j) * G
                ab = ((sloc * 2 + 0) * KT + j) * M6 + ro
                cb = ((sloc * 2 + 1) * KT + j) * M6 + ro
                l3w[:, ab:ab + A] = Wa3[skill]
                l3w[:, cb + A] = Wc3[skill, :, 0]
        in_maps.append({"xin": xin, "l1w": l1w, "l2w": l2w,
                        "l2b": l2b, "l3w": l3w})

    res = run_bass_kernel_spmd(nc, in_maps, core_ids=list(range(NCORES)))

    logits = np.empty((B, A), np.float32)
    value = np.empty((B,), np.float32)
    for c in range(NCORES):
        ot = res.results[c]["out"]  # [M6, len(spans), TILE]
        per = np.empty((2 * C, G), np.float32)
        for sloc in range(2):
            for sp, (off, W) in enumerate(spans):
                for j, (to, tw) in enumerate(_tiles_for(W)):
                    r0 = (sloc * KT + j) * G
                    blk = ot[r0:r0 + G, sp, :tw]  # [G, tw]
                    t0 = sloc * C + off + to
                    per[t0:t0 + tw] = blk.T
        for sloc in range(2):
            skill = 2 * c + sloc
            cnt = int(counts[skill])
            if cnt == 0:
                continue
            toks = order[starts[skill]:starts[skill] + cnt]
            blk = per[sloc * C:sloc * C + cnt]
            logits[toks] = blk[:, 0:A] + ba3[skill]
            value[toks] = blk[:, A] + bc3[skill, 0]
    return logits, value
